# revision 33
# baseline (speedup 1.0000x reference)
"""Transformer block (pre-LN causal MHA + GELU MLP) on 8 trn2 NeuronCores.

Sharding: core r handles batch b=r//4, group position p=r%4, owning token
chunks {p, 7-p} of eight 256-token chunks (causally balanced zigzag).
Everything is sequence-parallel except attention: K^T and V for the full
batch are exchanged via AllGathers inside each 4-core batch group, split
into two key-halves.

Attention computes transposed scores S^T[k, q] = K.Q^T so the softmax
row-sum falls out of a ones-augmented V matmul; no running max is needed
(|scores| <~ 7.7 for LN'd activations; exp is computed with a -3 shift so
the fp8 exp values stay in e4m3 range). Causal masks are applied by
accumulating -1e9 mask tiles into the score PSUM via identity matmuls.
Phases A (keys 0:1024) and B (keys 1024:2048) are interleaved per head
pair so chunk-B scores accumulate across phases in PSUM (no DRAM stash).

Precision: LN + residual stream fp32; QKV, attention AV and the output
projection run fp8e4 (e4m3) with DoubleRow matmuls (2x PE rate, fp32 PSUM
accumulation); scores run bf16; the MLP runs bf16 (fp8 exceeds the error
budget there). Host-side scales: W_attn x16, V x8, W_o x8, unscaled in
the bias/residual fused ops. Measured end-to-end relative error ~4e-3.

Self-contained: hardcodes B=2, T=2048, C=1024, H=16, D=64, hidden=4096.
"""
import sys

if "/opt/trn_rl_repo" not in sys.path:
    sys.path.insert(0, "/opt/trn_rl_repo")

import numpy as np
import ml_dtypes

B, T, C, H = 2, 2048, 1024, 16
D = C // H            # 64
MH = 4 * C            # 4096 mlp hidden
EPS = 1e-5
P = 128
TOK = 512             # tokens per core
NCH = 256             # tokens per chunk
N_CORES = 8
SCALE = 1.0 / np.sqrt(D)
SHIFT = 3.0           # exp(score - SHIFT) keeps et <= e^4.7 ~ 110 < 240 (e4m3 max)
WA_S = 16.0           # host scale on W_attn (fp8 storage)
V_S = 8.0             # scale on V (folded: v_sb = ps * (V_S/WA_S) + V_S*b_v)
WO_S = 8.0            # host scale on W_o; proj psum = V_S*WO_S * (y @ W_o)

_CACHE: dict = {}


def _build(mock_cc=False):
    import concourse.tile as tile
    from concourse import bacc, mybir
    from concourse.masks import make_identity
    from contextlib import ExitStack

    F32 = mybir.dt.float32
    BF16 = mybir.dt.bfloat16
    FP8 = mybir.dt.float8e4
    I32 = mybir.dt.int32
    AF = mybir.ActivationFunctionType
    ALU = mybir.AluOpType
    DR = mybir.MatmulPerfMode.DoubleRow

    nc = bacc.Bacc()

    # ---------------- I/O ----------------
    x_in = nc.declare_dram_parameter("x", [TOK, C], F32, isOutput=False)
    qbase_in = nc.declare_dram_parameter("qbase", [1, 2], F32, isOutput=False)
    w_attn = nc.declare_dram_parameter("w_attn", [C, 3 * C], FP8, isOutput=False)
    b_attn = nc.declare_dram_parameter("b_attn", [3 * C], F32, isOutput=False)
    w_o = nc.declare_dram_parameter("w_o", [C, C], FP8, isOutput=False)
    b_o = nc.declare_dram_parameter("b_o", [C], F32, isOutput=False)
    w_fc = nc.declare_dram_parameter("w_fc", [C, MH], BF16, isOutput=False)
    b_fc = nc.declare_dram_parameter("b_fc", [MH], F32, isOutput=False)
    w_fc2 = nc.declare_dram_parameter("w_fc2", [MH, C], BF16, isOutput=False)
    b_fc2 = nc.declare_dram_parameter("b_fc2", [C], F32, isOutput=False)
    out_ext = nc.declare_dram_parameter("out", [TOK, C], F32, isOutput=True)

    # internal DRAM for the collectives (A = keys 0:1024, B = keys 1024:2048)
    kt_in = [nc.dram_tensor(f"kt_in_{s}", [C, NCH], FP8) for s in range(2)]
    v_in = [nc.dram_tensor(f"v_in_{s}", [NCH, C], FP8) for s in range(2)]
    kt_all = [nc.dram_tensor(f"kt_all_{s}", [4 * C, NCH], FP8) for s in range(2)]
    v_all = [nc.dram_tensor(f"v_all_{s}", [4 * NCH, C], FP8) for s in range(2)]
    RG = [[0, 1, 2, 3], [4, 5, 6, 7]]

    with tile.TileContext(nc) as tc, ExitStack() as ctx:
        # ---------- pools: outer (whole kernel) ----------
        const = ctx.enter_context(tc.tile_pool(name="const", bufs=1))
        outer = ctx.enter_context(tc.tile_pool(name="outer", bufs=1))
        sm = ctx.enter_context(tc.tile_pool(name="sm", bufs=2))
        wmp = ctx.enter_context(tc.tile_pool(name="wmlp", bufs=3))

        # ---------- x load first (longest pole into LN1), split across queues ----------
        x_sb = outer.tile([P, 4, C], F32)
        for t in range(4):
            (nc.sync if t % 2 == 0 else nc.scalar).dma_start(
                out=x_sb[:, t, :], in_=x_in[P * t:P * (t + 1), :])

        # ---------- constants ----------
        ident = const.tile([P, P], F32)
        make_identity(nc, ident)
        eps_t = const.tile([P, 1], F32)
        nc.vector.memset(eps_t, EPS)
        shift_t = const.tile([P, 1], F32)
        nc.vector.memset(shift_t, -SHIFT)
        ident_bf = const.tile([P, P], BF16)
        nc.vector.tensor_copy(out=ident_bf, in_=ident)
        # pre-warm the sqrt activation table while x streams in
        dummy = const.tile([P, 1], F32)
        nc.scalar.activation(out=dummy, in_=eps_t, func=AF.Sqrt, scale=1.0)

        # per-feature bias tiles [128, f] views (loads deferred so the
        # scalar queue stays clear for the LN1 critical path)
        bq_sb = const.tile([P, 8], F32)     # b_attn[0:1024] (host-scaled by SCALE)
        bk_sb = const.tile([P, 8], F32)
        bfc_sb = const.tile([P, 32], F32)
        bv_bc = const.tile([P, C], F32)     # host-scaled by V_S
        bo_bc = const.tile([P, C], F32)
        b2_bc = const.tile([P, C], F32)

        def load_biases():
            nc.scalar.dma_start(out=bq_sb, in_=b_attn[0:C].rearrange("(f p) -> p f", p=P))
            nc.scalar.dma_start(out=bk_sb, in_=b_attn[C:2 * C].rearrange("(f p) -> p f", p=P))
            nc.scalar.dma_start(out=bfc_sb, in_=b_fc[:].rearrange("(f p) -> p f", p=P))
            nc.scalar.dma_start(out=bv_bc, in_=b_attn[2 * C:3 * C].rearrange("(a c) -> a c", a=1).to_broadcast((P, C)))
            nc.scalar.dma_start(out=bo_bc, in_=b_o[:].rearrange("(a c) -> a c", a=1).to_broadcast((P, C)))
            nc.scalar.dma_start(out=b2_bc, in_=b_fc2[:].rearrange("(a c) -> a c", a=1).to_broadcast((P, C)))

        # qbase + iotas for mask building
        qbase_sb = const.tile([1, 2], F32)
        nc.sync.dma_start(out=qbase_sb, in_=qbase_in[:, :])
        kidx_i = const.tile([P, 1], I32)
        nc.gpsimd.iota(kidx_i, pattern=[[0, 1]], base=0, channel_multiplier=1)
        kidx_f = const.tile([P, 1], F32)
        nc.vector.tensor_copy(out=kidx_f, in_=kidx_i)
        qio_i = const.tile([1, NCH], I32)
        nc.gpsimd.iota(qio_i, pattern=[[1, NCH]], base=0, channel_multiplier=0)
        qio_f = const.tile([1, NCH], F32)
        nc.vector.tensor_copy(out=qio_f, in_=qio_i)
        # qk[qc][k, q] = qglobal(qc, q) - k   (before subtracting 128*ktg)
        qk = []
        for qc in range(2):
            qg = const.tile([1, NCH], F32, name=f"qg{qc}")
            nc.vector.tensor_scalar_add(out=qg, in0=qio_f, scalar1=qbase_sb[0:1, qc:qc + 1])
            qgb = const.tile([P, NCH], F32, name=f"qgb{qc}")
            nc.gpsimd.partition_broadcast(qgb, qg)
            qkt = const.tile([P, NCH], F32, name=f"qk{qc}")
            nc.vector.tensor_scalar_sub(out=qkt, in0=qgb, scalar1=kidx_f)
            qk.append(qkt)
        # mask tiles M[k, q] = -1e9 where causally dead (qg < kg), else 0.
        # Phase A masks chunk 0 against kt 0..7; phase B masks chunk 1
        # against kt 8..15. Accumulated into score PSUM via identity matmul.
        mA = []
        for kt in range(8):
            m = const.tile([P, NCH], BF16, name=f"mA{kt}")
            nc.vector.tensor_scalar(out=m, in0=qk[0], scalar1=float(P * kt),
                                    scalar2=-1.0e9, op0=ALU.is_lt, op1=ALU.mult)
            mA.append(m)
        mB = []
        for kt in range(8):
            m = const.tile([P, NCH], BF16, name=f"mB{kt}")
            nc.vector.tensor_scalar(out=m, in0=qk[1], scalar1=float(P * (8 + kt)),
                                    scalar2=-1.0e9, op0=ALU.is_lt, op1=ALU.mult)
            mB.append(m)

        # ---------- helpers ----------
        def layer_norm(src, dst_pool, tag, dt=F32, on_act=False):
            # rstd = exp(-0.5*ln(var+eps)) keeps ACT on the exp/ln table
            # (no Sqrt table switch); with on_act the normalize itself runs
            # on ACT as Identity(x*rstd + (-mu*rstd)).
            ln = dst_pool.tile([P, 4, C], dt, name=tag, tag=tag)
            for t in range(4):
                stats = sm.tile([P, 2, 6], F32, name="lnstats", tag="lnstats")
                nc.vector.bn_stats(out=stats[:, 0, :], in_=src[:, t, 0:512])
                nc.vector.bn_stats(out=stats[:, 1, :], in_=src[:, t, 512:1024])
                mv = sm.tile([P, 2], F32, name="lnmv", tag="lnmv")
                nc.vector.bn_aggr(out=mv, in_=stats)
                rstd = sm.tile([P, 1], F32, name="lnrstd", tag="lnrstd")
                nc.scalar.activation(out=rstd, in_=mv[:, 1:2], func=AF.Sqrt, bias=eps_t, scale=1.0)
                nc.vector.reciprocal(out=rstd, in_=rstd)
                if on_act:
                    negmur = sm.tile([P, 1], F32, name="negmur", tag="negmur")
                    nc.vector.tensor_scalar(out=negmur, in0=mv[:, 0:1], scalar1=rstd,
                                            scalar2=-1.0, op0=ALU.mult, op1=ALU.mult)
                    nc.scalar.activation(out=ln[:, t, :], in_=src[:, t, :],
                                         func=AF.Identity, bias=negmur, scale=rstd)
                else:
                    nc.vector.tensor_scalar(out=ln[:, t, :], in0=src[:, t, :],
                                            scalar1=mv[:, 0:1], scalar2=rstd,
                                            op0=ALU.subtract, op1=ALU.mult)
            return ln

        def transpose_to(lnt, dst_pool, dst_tag, dt, idn):
            xt = dst_pool.tile([P, 8, TOK], dt, name=dst_tag, tag=dst_tag)
            with tc.tile_pool(name="tp_ps", bufs=2, space="PSUM") as tp_ps:
                for f in range(8):
                    for t in range(4):
                        pt = tp_ps.tile([P, P], lnt.dtype, name="tpt", tag="tpt",
                                        padded_shape=[P, 2 * P])
                        nc.tensor.transpose(pt[:, :], lnt[:, t, P * f:P * (f + 1)], idn)
                        nc.vector.tensor_copy(out=xt[:, f, P * t:P * (t + 1)], in_=pt[:, :])
            return xt

        x2 = outer.tile([P, 4, C], F32)

        with tc.tile_pool(name="mid", bufs=1) as mid:
            qT = mid.tile([P, 8, TOK], BF16)
            yT = mid.tile([P, 8, TOK], FP8)

            # ================= qkv =================
            with tc.tile_pool(name="qkvp", bufs=1) as qp, \
                 tc.tile_pool(name="wqkv", bufs=2) as wp, \
                 tc.tile_pool(name="qkv_ps", bufs=4, space="PSUM") as qkv_ps:
                ln1 = layer_norm(x_sb, qp, "ln", dt=BF16, on_act=True)
                xlnT = transpose_to(ln1, qp, "xlnT", dt=FP8, idn=ident_bf)
                load_biases()

                # K^T feature tiles -> kt_in halves
                for f in range(8):
                    if f % 4 == 0:
                        wk = wp.tile([P, 8, 512], FP8, name="wk", tag="wk")
                        nc.gpsimd.dma_start(out=wk, in_=w_attn[:, C + 512 * (f // 4): C + 512 * (f // 4 + 1)]
                                            .rearrange("(kc kp) n -> kp kc n", kp=P))
                    fo = P * (f % 4)
                    ps = qkv_ps.tile([P, TOK], F32, name="kps", tag="qkvps")
                    for i in range(4):
                        nc.tensor.matmul(ps[:, :], wk[:, 2 * i:2 * i + 2, fo:fo + P],
                                         xlnT[:, 2 * i:2 * i + 2, :],
                                         start=(i == 0), stop=(i == 3), perf_mode=DR)
                    kt_sb = sm.tile([P, TOK], FP8, name="kt_sb", tag="kt_sb", bufs=2)
                    nc.scalar.activation(out=kt_sb, in_=ps[:, :], func=AF.Identity,
                                         bias=bk_sb[:, f:f + 1], scale=1.0 / WA_S)
                    for s in range(2):
                        nc.sync.dma_start(out=kt_in[s][P * f:P * (f + 1), :],
                                          in_=kt_sb[:, NCH * s:NCH * (s + 1)])
                # V token tiles -> v_in halves (t-outer so the phase-A half
                # finishes after t=1 and the first AllGather can fire early)
                wvs = []
                for n in range(2):
                    wv = qp.tile([P, 8, 512], FP8, name=f"wv{n}", tag=f"wv{n}")
                    nc.gpsimd.dma_start(out=wv, in_=w_attn[:, 2 * C + 512 * n:2 * C + 512 * (n + 1)]
                                        .rearrange("(kc kp) n -> kp kc n", kp=P))
                    wvs.append(wv)
                for t in range(4):
                    for n in range(2):
                        ps = qkv_ps.tile([P, 512], F32, name="vps", tag="qkvps")
                        for i in range(4):
                            nc.tensor.matmul(ps[:, :], xlnT[:, 2 * i:2 * i + 2, P * t:P * (t + 1)],
                                             wvs[n][:, 2 * i:2 * i + 2, :],
                                             start=(i == 0), stop=(i == 3), perf_mode=DR)
                        v_sb = sm.tile([P, 512], FP8, name="v_sb", tag="v_sb")
                        nc.vector.scalar_tensor_tensor(out=v_sb, in0=ps[:, :], scalar=V_S / WA_S,
                                                       in1=bv_bc[:, 512 * n:512 * (n + 1)],
                                                       op0=ALU.mult, op1=ALU.add)
                        sh, row = divmod(t, 2)
                        nc.sync.dma_start(out=v_in[sh][P * row:P * (row + 1), 512 * n:512 * (n + 1)],
                                          in_=v_sb)
                # prefetch Q weights on the pool queue before the collectives
                # block it
                wqs = []
                for n in range(2):
                    wq = qp.tile([P, 8, 512], FP8, name=f"wq{n}", tag=f"wq{n}")
                    nc.gpsimd.dma_start(out=wq, in_=w_attn[:, 512 * n: 512 * (n + 1)]
                                        .rearrange("(kc kp) n -> kp kc n", kp=P))
                    wqs.append(wq)

                # collectives (gpsimd-triggered; overlap with Q^T compute below)
                for s in range(2):
                    if mock_cc:
                        half = C // 2
                        nc.gpsimd.dma_start(out=kt_all[s][0:half, :], in_=kt_in[s][0:half, :])
                        nc.scalar.dma_start(out=kt_all[s][half:C, :], in_=kt_in[s][half:C, :])
                        nc.gpsimd.dma_start(out=v_all[s][0:NCH // 2, :], in_=v_in[s][0:NCH // 2, :])
                        nc.scalar.dma_start(out=v_all[s][NCH // 2:NCH, :], in_=v_in[s][NCH // 2:NCH, :])
                    else:
                        nc.gpsimd.collective_compute("AllGather", ALU.bypass,
                                                     ins=[kt_in[s][:, :]], outs=[kt_all[s][:, :]],
                                                     replica_groups=RG)
                        nc.gpsimd.collective_compute("AllGather", ALU.bypass,
                                                     ins=[v_in[s][:, :]], outs=[v_all[s][:, :]],
                                                     replica_groups=RG)

                # Q^T feature tiles (stay local); fold in 1/sqrt(d) (host folded
                # the SCALE into b_attn[0:C])
                for f in range(8):
                    wq = wqs[f // 4]
                    fo = P * (f % 4)
                    ps = qkv_ps.tile([P, TOK], F32, name="qps", tag="qkvps")
                    for i in range(4):
                        nc.tensor.matmul(ps[:, :], wq[:, 2 * i:2 * i + 2, fo:fo + P],
                                         xlnT[:, 2 * i:2 * i + 2, :],
                                         start=(i == 0), stop=(i == 3), perf_mode=DR)
                    nc.scalar.activation(out=qT[:, f, :], in_=ps[:, :], func=AF.Identity,
                                         bias=bq_sb[:, f:f + 1], scale=SCALE / WA_S)

            # ============ attention: phases interleaved per head pair ============
            with tc.tile_pool(name="attp", bufs=1) as ap, \
                 tc.tile_pool(name="projp", bufs=1) as pp:
                wo_sb = pp.tile([P, 8, C], FP8)
                for t in range(4):
                    nc.vector.tensor_tensor(out=x_sb[:, t, :], in0=x_sb[:, t, :], in1=bo_bc, op=ALU.add)

                def load_kv(s):
                    ktb = ap.tile([P, 8, 4, NCH], FP8, name=f"ktb{s}", tag=f"ktb{s}")
                    # vb layout for DoubleRow: [part, pair, head, kt-in-pair, D+1]
                    vb = ap.tile([P, 4, 16, 2, D + 1], FP8, name=f"vb{s}", tag=f"vb{s}")
                    engs = [nc.sync, nc.scalar]
                    for r in range(4):
                        blk = r if s == 0 else 3 - r     # rank block -> key slot
                        engs[r % 2].dma_start(
                            out=ktb[:, :, blk, :],
                            in_=kt_all[s][C * r:C * (r + 1), :].rearrange("(j p) c -> p j c", p=P))
                        for sub in range(2):
                            kt = 2 * blk + sub
                            engs[(r + sub) % 2].dma_start(
                                out=vb[:, kt // 2, :, kt % 2, 0:D],
                                in_=v_all[s][NCH * r + P * sub:NCH * r + P * (sub + 1), :]
                                        .rearrange("p (h d) -> p h d", h=H))
                    nc.vector.memset(vb[:, :, :, :, D:D + 1], 1.0)
                    return ktb, vb

                ktb0, vb0 = load_kv(0)
                ktb1, vb1 = load_kv(1)
                nc.gpsimd.dma_start(out=wo_sb, in_=w_o[:, :].rearrange("(kc kp) n -> kp kc n", kp=P))

                # prefetch the first MLP weight tiles during attention
                wfc_pre = wmp.tile([P, 8, 512], BF16, name="wfc", tag="wfc", bufs=2)
                nc.sync.dma_start(out=wfc_pre, in_=w_fc[:, 0:512]
                                  .rearrange("(kc kp) n -> kp kc n", kp=P))
                w2_pre = wmp.tile([P, 4, 512], BF16, name="w2", tag="w2", bufs=3)
                nc.gpsimd.dma_start(out=w2_pre, in_=w_fc2[0:4 * P, 0:512]
                                    .rearrange("(mc mp) n -> mp mc n", mp=P))

                # lag-1 software pipeline: A(j) runs before B(j-1) so j=0's
                # phase B never head-of-line-blocks the PE on the second
                # gather, and exp(A) overlaps B matmuls.
                with tc.tile_pool(name="at_ps", bufs=1, space="PSUM") as at_ps:
                    ya_of = {}

                    def phase_A(j):
                        ya = [at_ps.tile([D + 1, TOK], F32, name=f"ya{h}", tag=f"ya{h}", bufs=2)
                              for h in range(2)]
                        ya_of[j] = ya

                        def av(pair, et):
                            for h in range(2):
                                nc.tensor.matmul(ya[h][:, :], vb0[:, pair, 2 * j + h, :, :],
                                                 et[:, :, h, :], perf_mode=DR,
                                                 start=(pair == 0), stop=False,
                                                 skip_group_check=True)

                        ets = []
                        for pair in range(4):
                            etp = sm.tile([P, 2, 2, TOK], FP8, name="etA", tag="etA", bufs=3)
                            for i in range(2):
                                kt = 2 * pair + i
                                st = at_ps.tile([P, 2, TOK], F32, name="st", tag="st", bufs=2)
                                for h in range(2):
                                    nc.tensor.matmul(
                                        st[:, h, :],
                                        ktb0[64 * h:64 * (h + 1), j, pair, i * P:(i + 1) * P],
                                        qT[64 * h:64 * (h + 1), j, :],
                                        start=True, stop=True, tile_position=(64 * h, 0))
                                for h in range(2):
                                    nc.tensor.matmul(st[:, h, 0:NCH], ident_bf, mA[kt],
                                                     start=False, stop=True,
                                                     skip_group_check=True)
                                nc.scalar.activation(out=etp[:, i, :, :], in_=st[:, :, :],
                                                     func=AF.Exp, bias=shift_t, scale=1.0)
                            ets.append(etp)
                            # AV lags one pair so the exp never head-of-line
                            # blocks the next pair's score matmuls on the PE
                            if pair >= 1:
                                av(pair - 1, ets[pair - 1])
                        av(3, ets[3])

                    def phase_B_div(j):
                        ya = ya_of.pop(j)

                        def av(pair, et):
                            for h in range(2):
                                nc.tensor.matmul(ya[h][:, NCH:TOK], vb1[:, pair, 2 * j + h, :, :],
                                                 et[:, :, h, :], perf_mode=DR,
                                                 start=False, stop=(pair == 3),
                                                 skip_group_check=True)

                        ets = []
                        for pair in range(4):
                            etb = sm.tile([P, 2, 2, NCH], FP8, name="etB", tag="etB", bufs=3)
                            stb = at_ps.tile([P, 2, 2, NCH], F32, name="st", tag="st", bufs=2)
                            for i in range(2):
                                kt = 2 * pair + i
                                for h in range(2):
                                    nc.tensor.matmul(
                                        stb[:, i, h, :],
                                        ktb1[64 * h:64 * (h + 1), j, pair, i * P:(i + 1) * P],
                                        qT[64 * h:64 * (h + 1), j, NCH:TOK],
                                        start=True, stop=True, tile_position=(64 * h, 0))
                                for h in range(2):
                                    nc.tensor.matmul(stb[:, i, h, :], ident_bf, mB[kt],
                                                     start=False, stop=True,
                                                     skip_group_check=True)
                            nc.scalar.activation(out=etb, in_=stb,
                                                 func=AF.Exp, bias=shift_t, scale=1.0)
                            ets.append(etb)
                            if pair >= 1:
                                av(pair - 1, ets[pair - 1])
                        av(3, ets[3])
                        # normalize: yT = ya[0:D] / ya[D] (guarded against den=0)
                        for h in range(2):
                            recip = sm.tile([1, TOK], F32, name=f"rc{h}", tag=f"rc{h}")
                            nc.vector.reciprocal(out=recip, in_=ya[h][D:D + 1, :])
                            nc.vector.tensor_scalar_min(out=recip, in0=recip, scalar1=1.0e30)
                            rb = sm.tile([D, TOK], F32, name=f"rb{h}", tag=f"rb{h}")
                            nc.gpsimd.partition_broadcast(rb, recip)
                            nc.vector.tensor_tensor(out=yT[64 * h:64 * (h + 1), j, :],
                                                    in0=ya[h][0:D, :], in1=rb, op=ALU.mult)

                    for jj in range(9):
                        if jj < 8:
                            phase_A(jj)
                        if jj >= 1:
                            phase_B_div(jj - 1)

                # ---- output projection (fp8 DoubleRow) + residual ----
                with tc.tile_pool(name="pr_ps", bufs=2, space="PSUM") as pr_ps:
                    for t in range(4):
                        for n in range(2):
                            ps = pr_ps.tile([P, 512], F32, name="prps", tag="prps")
                            for i in range(4):
                                nc.tensor.matmul(ps[:, :], yT[:, 2 * i:2 * i + 2, P * t:P * (t + 1)],
                                                 wo_sb[:, 2 * i:2 * i + 2, 512 * n:512 * (n + 1)],
                                                 start=(i == 0), stop=(i == 3), perf_mode=DR)
                            nc.vector.scalar_tensor_tensor(out=x2[:, t, 512 * n:512 * (n + 1)],
                                                           in0=ps[:, :], scalar=1.0 / (V_S * WO_S),
                                                           in1=x_sb[:, t, 512 * n:512 * (n + 1)],
                                                           op0=ALU.mult, op1=ALU.add)

        # ================= LN2 + MLP (bf16) =================
        with tc.tile_pool(name="mlpp", bufs=1) as mp:
            ln2 = layer_norm(x2, mp, "ln2", dt=BF16, on_act=True)
            xln2T = transpose_to(ln2, mp, "xln2T", dt=BF16, idn=ident_bf)
            for t in range(4):
                nc.vector.tensor_tensor(out=x2[:, t, :], in0=x2[:, t, :], in1=b2_bc, op=ALU.add)

            h_sb = mp.tile([P, 32, 512], BF16)
            for half in range(2):
                with tc.tile_pool(name=f"mlp_ps{half}", bufs=1, space="PSUM") as mlp_ps:
                    ops = [mlp_ps.tile([P, 512], F32, name=f"ops{t}", tag=f"ops{t}", bufs=1)
                           for t in range(4)]
                    for m in range(32):
                        if half == 0:
                            if m % 4 == 0:
                                if m == 0:
                                    wfc = wfc_pre
                                else:
                                    wfc = wmp.tile([P, 8, 512], BF16, name="wfc", tag="wfc", bufs=2)
                                    nc.sync.dma_start(out=wfc,
                                                      in_=w_fc[:, 512 * (m // 4):512 * (m // 4 + 1)]
                                                      .rearrange("(kc kp) n -> kp kc n", kp=P))
                            mo = P * (m % 4)
                            fps = mlp_ps.tile([P, 512], F32, name="fps", tag="fps", bufs=4)
                            for k in range(8):
                                nc.tensor.matmul(fps[:, :], wfc[:, k, mo:mo + P], xln2T[:, k, :],
                                                 start=(k == 0), stop=(k == 7))
                            nc.scalar.activation(out=h_sb[:, m, :], in_=fps[:, :], func=AF.Gelu,
                                                 bias=bfc_sb[:, m:m + 1], scale=1.0)
                        if m % 4 == 0:
                            if half == 0 and m == 0:
                                w2 = w2_pre
                            else:
                                w2 = wmp.tile([P, 4, 512], BF16, name="w2", tag="w2", bufs=3)
                                nc.gpsimd.dma_start(out=w2, in_=w_fc2[P * m:P * (m + 4),
                                                                   512 * half:512 * (half + 1)]
                                                    .rearrange("(mc mp) n -> mp mc n", mp=P))
                        for t in range(4):
                            nc.tensor.matmul(ops[t][:, :], h_sb[:, m, P * t:P * (t + 1)],
                                             w2[:, m % 4, :], start=(m == 0), stop=(m == 31))
                    for t in range(4):
                        nc.vector.tensor_tensor(out=x2[:, t, 512 * half:512 * (half + 1)],
                                                in0=ops[t][:, :],
                                                in1=x2[:, t, 512 * half:512 * (half + 1)], op=ALU.add)
                        if half == 1:
                            nc.sync.dma_start(out=out_ext[P * t:P * (t + 1), :], in_=x2[:, t, :])

    nc.finalize()
    return nc


def _get_nc():
    if "nc" not in _CACHE:
        _CACHE["nc"] = _build()
    return _CACHE["nc"]


def _prep(**inputs):
    f = lambda a: np.asarray(a, dtype=np.float32)
    x = f(inputs["x"])
    ln1_g, ln1_b = f(inputs["ln1_g"]), f(inputs["ln1_b"])
    ln2_g, ln2_b = f(inputs["ln2_g"]), f(inputs["ln2_b"])
    W_attn, b_attn = f(inputs["W_attn"]), f(inputs["b_attn"])
    W_o, b_o = f(inputs["W_o"]), f(inputs["b_o"])
    W_fc, b_fc = f(inputs["W_fc"]), f(inputs["b_fc"])
    W_fc2, b_fc2 = f(inputs["W_fc2"]), f(inputs["b_fc2"])

    # fold LN affine params into the next matmul
    W_attn_e = ln1_g[:, None] * W_attn
    b_attn_e = b_attn + ln1_b @ W_attn
    W_fc_e = ln2_g[:, None] * W_fc
    b_fc_e = b_fc + ln2_b @ W_fc

    # fp8 storage scales + per-segment bias scales (see kernel docstring)
    w_attn_q = (W_attn_e * WA_S).astype(ml_dtypes.float8_e4m3)
    b_attn_q = np.concatenate([b_attn_e[0:C] * SCALE,
                               b_attn_e[C:2 * C],
                               b_attn_e[2 * C:3 * C] * V_S]).astype(np.float32)
    w_o_q = (W_o * WO_S).astype(ml_dtypes.float8_e4m3)

    in_maps = []
    for r in range(N_CORES):
        b, p = divmod(r, 4)
        c0, c1 = p, 7 - p
        xs = np.concatenate([x[b, NCH * c0:NCH * (c0 + 1)],
                             x[b, NCH * c1:NCH * (c1 + 1)]], axis=0)
        in_maps.append({
            "x": np.ascontiguousarray(xs),
            "qbase": np.array([[NCH * c0, NCH * c1]], dtype=np.float32),
            "w_attn": w_attn_q, "b_attn": b_attn_q,
            "w_o": w_o_q, "b_o": b_o,
            "w_fc": W_fc_e.astype(ml_dtypes.bfloat16), "b_fc": b_fc_e,
            "w_fc2": W_fc2.astype(ml_dtypes.bfloat16), "b_fc2": b_fc2,
        })

    def assemble(results):
        out = np.empty((B, T, C), dtype=np.float32)
        for r in range(N_CORES):
            b, p = divmod(r, 4)
            c0, c1 = p, 7 - p
            o = results[r]["out"]
            out[b, NCH * c0:NCH * (c0 + 1)] = o[0:NCH]
            out[b, NCH * c1:NCH * (c1 + 1)] = o[NCH:TOK]
        return out

    return in_maps, assemble


def kernel(**inputs):
    from concourse.bass_utils import run_bass_kernel_spmd

    in_maps, assemble = _prep(**inputs)
    res = run_bass_kernel_spmd(_get_nc(), in_maps, list(range(N_CORES)))
    return assemble(res.results)
